# revision 21
# baseline (speedup 1.0000x reference)
"""Trainium2 Bass kernel for retrieval-KNN (nn_Bridge_39505109188914).

For each of 262144 query points in [0,1]^3: find the 8 nearest of 16384
anchors (squared euclidean), softmax(-d^2/0.005) over those 8, and return the
weighted sum of the anchors' 64-dim feature rows.

Data-parallel over 8 NeuronCores, 32768 queries each.  Per 128-query tile:
  - PE: M3 = q . p            (K=3 matmul, fma chain in x,y,z order)
        t  = |q|^2 + |p|^2    (K=2 matmul: qsq*1 + 1*psq -> one rounded add)
  - ACT: stage M3, t from PSUM to SBUF
  - Pool: S = (-2*M3) + t     (single rounding; bit-matches the reference's
        (qsq+psq) - 2*(q@pT) evaluation order)  -> V = S, selection by
        nc.vector.max on -S ... actually we keep S and select the 8 smallest
        via max on negated scale: we negate in the same op (see below).
  - DVE: nc.vector.max / max_index on V = -S per half + exact 16->8 merge
        (tie semantics identical to jax.lax.top_k: equal values resolved to
        increasing index order).
  - weights: softmax(-S/T) == softmax(V/T) on the 8 values (ACT exp).
  - SWDGE dma_gather of the 1024 feature rows, DVE weighted sum, DMA out.

kernel(**inputs) shards queries across 8 cores, runs the SPMD program,
returns the concatenated [262144, 64] output.
"""

import sys
import numpy as np

if "/opt/trn_rl_repo" not in sys.path:
    sys.path.insert(0, "/opt/trn_rl_repo")

K = 8
TEMP = 2.0 * 0.05 ** 2  # 0.005
N_CORES = 8

_prog_cache = {}


def build_program(b_core: int, n: int, f: int, n_cores: int = N_CORES,
                  with_idx: bool = True):
    """Emit the per-core bass program (identical on all cores)."""
    import concourse.bacc as bacc
    import concourse.mybir as mybir
    from concourse import tile

    assert b_core % 128 == 0 and n % 2048 == 0
    n2 = n // 2
    tiles = b_core // 128
    PCW = 2048 if n2 % 2048 == 0 else n2   # psum tile width
    CW = PCW                               # staging chunk width
    FP = mybir.dt.float32
    U16 = mybir.dt.uint16

    nc = bacc.Bacc("TRN2", target_bir_lowering=False, debug=False,
                   num_devices=n_cores, num_swdge_queues=4)
    # q rows: 0-2 = qx,qy,qz ; 3 = qsq ; 4 = ones
    q_dram = nc.declare_dram_parameter("q", [5, b_core], FP, isOutput=False)
    # posN (N=0,1 anchor half): rows 0-2 = px,py,pz ; 32 = ones ; 33 = psq
    pos0_dram = nc.declare_dram_parameter("pos0", [64, n2], FP, isOutput=False)
    pos1_dram = nc.declare_dram_parameter("pos1", [64, n2], FP, isOutput=False)
    feat_dram = nc.declare_dram_parameter("feat", [n, f], FP, isOutput=False)
    out_dram = nc.declare_dram_parameter("out", [b_core, f], FP, isOutput=True)
    if with_idx:
        idx_dram = nc.declare_dram_parameter("idx", [b_core, K], U16, isOutput=True)

    AOP = mybir.AluOpType

    with tile.TileContext(nc) as tc:
        with tc.tile_pool(name="persist", bufs=1) as persist, \
             tc.tile_pool(name="vpool", bufs=1) as vpool, \
             tc.tile_pool(name="stage", bufs=2) as stage, \
             tc.tile_pool(name="small", bufs=3) as small, \
             tc.tile_pool(name="psum", bufs=1, space="PSUM") as psum_pool:

            pos_sb0 = persist.tile([64, n2], FP)
            nc.sync.dma_start(out=pos_sb0[:, :], in_=pos0_dram[:, :])
            pos_sb1 = persist.tile([64, n2], FP)
            nc.sync.dma_start(out=pos_sb1[:, :], in_=pos1_dram[:, :])
            pos_sbs = [pos_sb0, pos_sb1]
            iota16 = persist.tile([128, 16], FP)
            nc.gpsimd.iota(iota16[:, :], pattern=[[1, 16]], base=0,
                           channel_multiplier=0,
                           allow_small_or_imprecise_dtypes=True)

            for t in range(tiles):
                qsl = q_dram[:, t * 128:(t + 1) * 128]
                qt = small.tile([64, 128], FP, tag="qt")
                nc.sync.dma_start(out=qt[0:3, :], in_=qsl[0:3, :])
                nc.sync.dma_start(out=qt[32:34, :], in_=qsl[3:5, :])

                catv = small.tile([128, 16], FP, tag="catv")
                cati = small.tile([128, 16], U16, tag="cati")

                for h in range(2):
                    Vh = vpool.tile([128, n2], FP, tag=f"V{h}")
                    psb = pos_sbs[h]
                    # rhs row pair for t: (32=ones, 33+h=psq half)
                    for pc in range(n2 // PCW):
                        mps = psum_pool.tile([128, PCW], FP, tag="mps")
                        tps = psum_pool.tile([128, PCW], FP, tag="tps")
                        for m in range(PCW // 512):
                            lcol = pc * PCW + m * 512
                            nc.tensor.matmul(
                                mps[:, m * 512:(m + 1) * 512],
                                lhsT=qt[0:3, :],
                                rhs=psb[0:3, lcol:lcol + 512],
                                start=True, stop=True)
                        for m in range(PCW // 512):
                            lcol = pc * PCW + m * 512
                            # pairing: (qsq<->ones) + (ones<->psq)
                            nc.tensor.matmul(
                                tps[:, m * 512:(m + 1) * 512],
                                lhsT=qt[32:34, :],
                                rhs=psb[32:34, lcol:lcol + 512],
                                start=True, stop=True)
                        # stage to SBUF in CW chunks, then fuse on Pool.
                        # V = -S = 2*M3 - t : single rounding, the exact
                        # negation of the reference's round(t - 2*M3).
                        for s in range(PCW // CW):
                            msb = stage.tile([128, CW], FP, tag="msb")
                            tsb = stage.tile([128, CW], FP, tag="tsb")
                            # stage with exact x2 on the ACT copy
                            nc.scalar.mul(msb[:, :], mps[:, s * CW:(s + 1) * CW], 2.0)
                            nc.scalar.copy(tsb[:, :], tps[:, s * CW:(s + 1) * CW])
                            nc.gpsimd.tensor_tensor(
                                out=Vh[:, pc * PCW + s * CW:pc * PCW + (s + 1) * CW],
                                in0=msb[:, :], in1=tsb[:, :], op=AOP.subtract)

                    nc.vector.max(out=catv[:, 8 * h:8 * h + 8], in_=Vh[:, :])
                    nc.vector.max_index(out=cati[:, 8 * h:8 * h + 8],
                                        in_max=catv[:, 8 * h:8 * h + 8],
                                        in_values=Vh[:, :])

                # h1 indices are local to the second half: +n2
                nc.vector.tensor_scalar(cati[:, 8:16], cati[:, 8:16], float(n2),
                                        None, AOP.add)
                # merge: global top8 values + positions within the 16
                comb8 = small.tile([128, 8], FP, tag="comb8")
                nc.vector.max(out=comb8[:, :], in_=catv[:, :])
                pos8 = small.tile([128, 8], U16, tag="pos8")
                nc.vector.max_index(out=pos8[:, :], in_max=comb8[:, :],
                                    in_values=catv[:, :])
                # sel_idx[k] = sum_j cati[j] * (pos8[k] == j)
                pos8f = small.tile([128, 8], FP, tag="pos8f")
                nc.vector.tensor_copy(pos8f[:, :], pos8[:, :])
                catif = small.tile([128, 16], FP, tag="catif")
                nc.vector.tensor_copy(catif[:, :], cati[:, :])
                oneh = small.tile([128, 8, 16], FP, tag="oneh")
                nc.vector.tensor_tensor(
                    out=oneh[:, :, :],
                    in0=pos8f.rearrange("p (k o) -> p k o", o=1).to_broadcast([128, 8, 16]),
                    in1=iota16.rearrange("p (o j) -> p o j", o=1).to_broadcast([128, 8, 16]),
                    op=AOP.is_equal)
                nc.vector.tensor_tensor(
                    out=oneh[:, :, :], in0=oneh[:, :, :],
                    in1=catif.rearrange("p (o j) -> p o j", o=1).to_broadcast([128, 8, 16]),
                    op=AOP.mult)
                selif = small.tile([128, 8], FP, tag="selif")
                nc.vector.tensor_reduce(selif[:, :], oneh[:, :, :],
                                        axis=mybir.AxisListType.X, op=AOP.add)
                sel = small.tile([128, 8], U16, tag="sel")
                nc.vector.tensor_copy(sel[:, :], selif[:, :])

                # softmax weights over the 8 (scale 1/T, stabilized by Vmax)
                nbias = small.tile([128, 1], FP, tag="nbias")
                nc.scalar.mul(nbias[:, :], comb8[:, 0:1], -1.0 / TEMP)
                ew = small.tile([128, 8], FP, tag="ew")
                ssum = small.tile([128, 1], FP, tag="ssum")
                nc.scalar.activation(ew[:, :], comb8[:, :],
                                     mybir.ActivationFunctionType.Exp,
                                     bias=nbias[:, 0:1], scale=1.0 / TEMP,
                                     accum_out=ssum[:, 0:1])
                rsum = small.tile([128, 1], FP, tag="rsum")
                nc.vector.reciprocal(rsum[:, :], ssum[:, :])
                w = small.tile([128, 8], FP, tag="w")
                nc.vector.tensor_scalar(w[:, :], ew[:, :], rsum[:, 0:1], None,
                                        AOP.mult)

                # wrap sel into SWDGE idx layout: list[j]=sel[q,k] at j=k*128+q
                # -> wrap[p, 8k+g] = sel[16g+p, k]   (p<16; rows 16.. zeroed)
                wrap = small.tile([128, 64], U16, tag="wrap")
                wrap_kg = wrap[0:16, :].rearrange("p (k g) -> p k g", k=8)
                for g in range(8):
                    nc.sync.dma_start(
                        out=wrap_kg[:, :, g:g + 1],
                        in_=sel[16 * g:16 * (g + 1), :].rearrange(
                            "p (k o) -> p k o", o=1))
                # replicate the wrapped list into the other 7 Q7 core groups
                for c in range(1, 8):
                    nc.sync.dma_start(out=wrap[16 * c:16 * (c + 1), :],
                                      in_=wrap[0:16, :])

                G = small.tile([128, 8, f], FP, tag="G")
                nc.gpsimd.dma_gather(
                    out_ap=G[:, :, :],
                    in_ap=feat_dram[:, :],
                    idxs_ap=wrap[:, :].bitcast(mybir.dt.int16),
                    num_idxs=128 * 8,
                    num_idxs_reg=128 * 8,
                    elem_size=f,
                    queue_num=t % 4)

                P = small.tile([128, 8, f], FP, tag="P")
                w_bc = w.rearrange("p (k o) -> p k o", o=1).to_broadcast([128, 8, f])
                nc.vector.tensor_mul(P[:, :, :], G[:, :, :], w_bc)
                acc = small.tile([128, f], FP, tag="acc")
                nc.vector.tensor_reduce(acc[:, :], P.rearrange("p k f -> p f k"),
                                        axis=mybir.AxisListType.X, op=AOP.add)
                nc.sync.dma_start(out=out_dram[t * 128:(t + 1) * 128, :],
                                  in_=acc[:, :])
                if with_idx:
                    nc.sync.dma_start(out=idx_dram[t * 128:(t + 1) * 128, :],
                                      in_=sel[:, :])

    nc.compile()
    return nc


def _prep_host(coords, positions, features, n_cores):
    """Host-side input prep: augmented transposes + query sharding."""
    B = coords.shape[0]
    n, f = features.shape
    n2 = n // 2
    b_core = B // n_cores

    c = coords.astype(np.float32)
    qsq = (c[:, 0] * c[:, 0] + c[:, 1] * c[:, 1]) + c[:, 2] * c[:, 2]
    q_aug = np.empty((5, B), dtype=np.float32)
    q_aug[0:3, :] = c.T
    q_aug[3, :] = qsq
    q_aug[4, :] = 1.0

    p = positions.astype(np.float32)
    psq = (p[:, 0] * p[:, 0] + p[:, 1] * p[:, 1]) + p[:, 2] * p[:, 2]
    def make_pos(sl):
        ps = np.zeros((64, n2), dtype=np.float32)
        ps[0:3, :] = p[sl].T
        ps[32, :] = 1.0
        ps[33, :] = psq[sl]
        return ps
    pos0 = make_pos(slice(0, n2))
    pos1 = make_pos(slice(n2, n))

    feats = np.ascontiguousarray(features.astype(np.float32))
    in_maps = []
    for ci in range(n_cores):
        in_maps.append({
            "q": np.ascontiguousarray(q_aug[:, ci * b_core:(ci + 1) * b_core]),
            "pos0": pos0,
            "pos1": pos1,
            "feat": feats,
        })
    return in_maps, b_core


def kernel(coords: np.ndarray, positions: np.ndarray, features: np.ndarray) -> np.ndarray:
    from concourse.bass_utils import run_bass_kernel_spmd

    coords = np.asarray(coords)
    positions = np.asarray(positions)
    features = np.asarray(features)
    B = coords.shape[0]
    n, f = features.shape
    b_core = B // N_CORES

    key = (b_core, n, f)
    if key not in _prog_cache:
        _prog_cache[key] = build_program(b_core, n, f)
    nc = _prog_cache[key]

    in_maps, _ = _prep_host(coords, positions, features, N_CORES)
    res = run_bass_kernel_spmd(nc, in_maps, list(range(N_CORES)))
    out = np.concatenate([res.results[i]["out"] for i in range(N_CORES)], axis=0)
    return out.astype(np.float32)


def kernel_with_idx(coords, positions, features):
    """Debug entry: returns (out, idx) with idx the selected anchor ids."""
    from concourse.bass_utils import run_bass_kernel_spmd
    B = coords.shape[0]
    n, f = features.shape
    b_core = B // N_CORES
    key = (b_core, n, f)
    if key not in _prog_cache:
        _prog_cache[key] = build_program(b_core, n, f)
    nc = _prog_cache[key]
    in_maps, _ = _prep_host(np.asarray(coords), np.asarray(positions),
                            np.asarray(features), N_CORES)
    res = run_bass_kernel_spmd(nc, in_maps, list(range(N_CORES)))
    out = np.concatenate([res.results[i]["out"] for i in range(N_CORES)], axis=0)
    idx = np.concatenate([res.results[i]["idx"] for i in range(N_CORES)], axis=0)
    return out.astype(np.float32), idx
